# revision 15
# baseline (speedup 1.0000x reference)
"""Trainium2 Bass kernel for a 2-layer GRU time-series binary classifier.

Model (torch GRU semantics, batch_first):
  seq1, _ = GRU(F=2048 -> H1=128)(x)        x: [64, 512, 2048]
  _,  h2 = GRU(H1 -> H2=64)(seq1)
  out = h2 @ fc_w.T + fc_b                  -> [64, 1]

Strategy: data-parallel over batch across 8 cores (8 sequences each).
The kernel is serial-chain bound (512 strictly sequential GRU steps), so
the design minimizes per-step chain latency:

  mm(r,z,n) -> sigmoid(r,z) -> scanA -> tanh -> scanB        (5 links)

using tensor_tensor_scan as a fused two-tensor multiply-add:
  scanA pairs: state=hnb; state=r*state+xn  => t2 = r*(hn+b_hhn)+xn
  scanB pairs: state=n;   state=omz*state+zh => h' = (1-z)*n + z*h
omz = sigmoid(-z_pre) comes from a second ACT op (scale=-1) and
zh = z*h runs on GPSIMD, both off the critical chain.  Both layers'
r/z/n pre-activations live in adjacent PSUM banks (L1 bank | L2 bank)
so every elementwise op covers L1 and L2 in ONE instruction via a
cross-bank access pattern; L2's matmuls use zero-padded stationaries so
its unused partitions 64-127 hold exact zeros.  All biases are folded
into PSUM by ones-matmul prefill, and per-step r/z recurrent matmuls
accumulate onto the chunked input-projection GEMM outputs in place.
"""

import numpy as np
import ml_dtypes

from concourse import bacc, tile, mybir
from concourse.bass_utils import run_bass_kernel_spmd

BF16 = ml_dtypes.bfloat16
N_CORES = 8
B, T, F = 64, 512, 2048
H1, H2 = 128, 64
B_LOC = B // N_CORES          # 8 sequences per core
CHUNK = 16                    # timesteps per GEMM chunk
NCH = T // CHUNK              # 32 chunks
LAG = 2 * CHUNK               # L2 runs 2 chunks behind L1
KT = F // 128                 # 16 K-tiles for GEMM1
NW = CHUNK * B_LOC            # 128 moving columns per chunk GEMM
NROUND = T + LAG              # 544 rounds
AF = mybir.ActivationFunctionType
ALU = mybir.AluOpType
DT_BF = mybir.dt.bfloat16
DT_F32 = mybir.dt.float32
SBUFS = 4                     # rotation depth for per-step tiles


def build_nc():
    nc = bacc.Bacc(None, target_bir_lowering=False)

    xT = nc.declare_dram_parameter("xT", [F, T, B_LOC], DT_BF, isOutput=False)
    wih1T = nc.declare_dram_parameter("wih1T", [F, 3 * H1], DT_BF, isOutput=False)
    whh1T = nc.declare_dram_parameter("whh1T", [H1, 3 * 128], DT_BF, isOutput=False)
    wih2T = nc.declare_dram_parameter("wih2T", [H1, 3 * 128], DT_BF, isOutput=False)
    whh2T = nc.declare_dram_parameter("whh2T", [H2, 3 * 128], DT_BF, isOutput=False)
    brow1 = nc.declare_dram_parameter("brow1", [1, 3 * 128], DT_BF, isOutput=False)
    brow2 = nc.declare_dram_parameter("brow2", [1, 3 * 128], DT_BF, isOutput=False)
    bn1row = nc.declare_dram_parameter("bn1row", [1, 128], DT_BF, isOutput=False)
    bn2row = nc.declare_dram_parameter("bn2row", [1, 128], DT_BF, isOutput=False)
    fcwT = nc.declare_dram_parameter("fcwT", [H2, 1], DT_BF, isOutput=False)
    fcb = nc.declare_dram_parameter("fcb", [B_LOC, 1], DT_F32, isOutput=False)
    out = nc.declare_dram_parameter("out", [B_LOC, 1], DT_F32, isOutput=True)

    with tile.TileContext(nc) as tc:
        with (
            tc.tile_pool(name="const", bufs=1) as cpool,
            tc.tile_pool(name="xchunk", bufs=3) as xpool,
            tc.tile_pool(name="psum", bufs=3, space="PSUM") as ppool,
        ):
            # ---- persistent tiles -------------------------------------
            w1 = cpool.tile([128, KT, 3 * H1], DT_BF)      # GEMM1 stationaries
            wh1 = cpool.tile([H1, 3 * 128], DT_BF)
            w2 = cpool.tile([H1, 3 * 128], DT_BF)          # zero-padded
            wh2 = cpool.tile([H2, 3 * 128], DT_BF)         # zero-padded
            br1 = cpool.tile([1, 3 * 128], DT_BF)
            br2 = cpool.tile([1, 3 * 128], DT_BF)
            bn1 = cpool.tile([1, 128], DT_BF)
            bn2 = cpool.tile([1, 128], DT_BF)
            ones = cpool.tile([1, NW], DT_BF)
            fw = cpool.tile([H2, 1], DT_BF)
            fb = cpool.tile([B_LOC, 1], DT_F32)
            res = cpool.tile([B_LOC, 1], DT_F32)
            # h history: per round-slot 32 cols = (layer:2) x (b:8) x
            # (junk|h:2).  h_t real values sit at odd cols.
            hh = cpool.tile([128, (NROUND + 2) * 32], DT_BF)

            nc.sync.dma_start(out=w1[:], in_=wih1T.rearrange("(kt p) g -> p kt g", p=128))
            nc.sync.dma_start(out=wh1[:], in_=whh1T[:])
            nc.sync.dma_start(out=w2[:], in_=wih2T[:])
            nc.sync.dma_start(out=wh2[:], in_=whh2T[:])
            nc.sync.dma_start(out=br1[:], in_=brow1[:])
            nc.sync.dma_start(out=br2[:], in_=brow2[:])
            nc.sync.dma_start(out=bn1[:], in_=bn1row[:])
            nc.sync.dma_start(out=bn2[:], in_=bn2row[:])
            nc.sync.dma_start(out=fw[:], in_=fcwT[:])
            nc.sync.dma_start(out=fb[:], in_=fcb[:])
            nc.vector.memset(ones[:], 1.0)

            hh_r = hh.rearrange("p (t l b w) -> p t l b w", l=2, b=B_LOC, w=2)
            # h_0 = 0 for L1 (slot 0) and L2 (slot LAG)
            nc.vector.memset(hh[:, 0:32], 0.0)
            nc.vector.memset(hh[:, LAG * 32:(LAG + 1) * 32], 0.0)

            # Persistent per-step tiles: the serial h-chain already orders
            # round t's readers before round t+1's writers, so one instance
            # suffices.  dz/d0B even columns are the scans' zero slots --
            # memset once; only odds are ever rewritten.
            dz = cpool.tile([128, 64], DT_F32)
            d0B = cpool.tile([128, 32], DT_F32)
            d1B = cpool.tile([128, 32], DT_F32)
            t2s = cpool.tile([128, 32], DT_F32)
            nc.vector.memset(dz[:], 0.0)
            nc.vector.memset(d0B[:], 0.0)
            dzr = dz.rearrange("p (g2 l b w) -> p l g2 b w", g2=2, l=2, w=2)
            d0Br = d0B.rearrange("p (l b w) -> p l b w", l=2, w=2)
            d1Br = d1B.rearrange("p (l b w) -> p l b w", l=2, w=2)
            t2sr = t2s.rearrange("p (l b w) -> p l b w", l=2, w=2)

            # ---- chunk-state ------------------------------------------
            xtiles = {}
            pts = {}     # chunk-round r -> psum tile [128, 1024] (L1|L2 banks)

            def alloc_pt(r):
                pts[r] = ppool.tile([128, 1024], DT_F32, tag="pt", name="pt")
                # All matmuls into pt use start=False and accumulate onto
                # explicit zeros (the sim's pending-zero model can't track
                # interleaved stride-2 dsts next to start=True banks).
                nc.vector.memset(pts[r][:], 0.0)

            def dma_xchunk(c):
                xt = xpool.tile([128, KT, NW], DT_BF, tag="xc", name="xc")
                nc.sync.dma_start(
                    out=xt[:],
                    in_=xT[:, c * CHUNK:(c + 1) * CHUNK, :].rearrange(
                        "(kt p) t b -> p kt (t b)", p=128),
                )
                xtiles[c] = xt

            # PSUM layout per chunk-round (pt = [128, 1024] = 2 banks):
            #   n  bank: L1 (hnb|xn) pairs [0:256] (col 16t+2b+w),
            #            L2 pairs [256:512]
            #   rz bank: L1r [512:640] (col 8t+b), L1z [640:768],
            #            L2r [768:896], L2z [896:1024]
            # GEMM dsts are contiguous or stride-2 2-D slices; the scans
            # read contiguous 16-col windows (scanA split per layer, both on
            # DVE: GPSIMD cannot access PSUM).
            def _rzv(c):
                return pts[c][:, 512:1024].rearrange(
                    "p (l g tb) -> p l g tb", l=2, g=2)

            def _nv(c, l):
                return pts[c][:, 256 * l:256 * l + 256].rearrange(
                    "p (j w) -> p j w", w=2)

            def gemm1_thunks(c):
                """Layer-1 input projection of chunk c.  Its first matmul
                into each bank carries start=True (clears has_written for
                the whole bank)."""
                xt = xtiles[c]
                pt = pts[c]
                nL1 = _nv(c, 0)
                thunks = []
                for g, dst in ((0, pt[:, 512:640]), (1, pt[:, 640:768]),
                               (2, nL1[:, :, 1])):
                    def mk(kt, g=g, dst=dst):
                        def f():
                            nc.tensor.matmul(
                                dst, w1[:, kt, g * 128:(g + 1) * 128], xt[:, kt],
                                start=False, stop=False,
                                skip_group_check=True)
                        return f
                    for kt in range(KT):
                        thunks.append(mk(kt))

                    def fbias(g=g, dst=dst):
                        nc.tensor.matmul(
                            dst, br1[:, g * 128:(g + 1) * 128], ones[:],
                            start=False, stop=(g == 2), skip_group_check=True)
                    thunks.append(fbias)

                def fpre():
                    nc.tensor.matmul(
                        nL1[:, :, 0], bn1[:], ones[:],
                        start=False, stop=False, skip_group_check=True)
                thunks.append(fpre)
                return thunks

            def gemm2_thunks(j, first):
                """Layer-2 input projection of its chunk j (consumes h1
                history rounds 16j..16j+15) into round-chunk (j+2).
                `first`: no gemm1 shares this round, so carry the bank
                clears here."""
                pt = pts[j + 2]
                nL2 = _nv(j + 2, 1)
                mv = hh_r[:, CHUNK * j + 1:CHUNK * j + 1 + CHUNK, 0, :, 1]
                thunks = []
                for g, dst in ((0, pt[:, 768:896]), (1, pt[:, 896:1024]),
                               (2, nL2[:, :, 1])):
                    def fmm(g=g, dst=dst):
                        nc.tensor.matmul(
                            dst, w2[:, g * 128:(g + 1) * 128], mv,
                            start=False, stop=False,
                            skip_group_check=True)
                    thunks.append(fmm)

                    def fbias(g=g, dst=dst):
                        nc.tensor.matmul(
                            dst, br2[:, g * 128:(g + 1) * 128], ones[:],
                            start=False, stop=(g == 2), skip_group_check=True)
                    thunks.append(fbias)

                def fpre():
                    nc.tensor.matmul(
                        nL2[:, :, 0], bn2[:], ones[:],
                        start=False, stop=False, skip_group_check=True)
                thunks.append(fpre)
                return thunks

            def round_step(s):
                """One merged GRU step for both layers at round s.
                L1 computes its step s; L2 computes its step s-LAG."""
                t = s % CHUNK
                c = s // CHUNK
                pt = pts[c]
                rzv = _rzv(c)
                lo, hi = (0, 2)
                if s < LAG:
                    lo, hi = 0, 1          # L1 only
                elif s >= T:
                    lo, hi = 1, 2          # L2 only

                # --- recurrent matmuls (accumulate onto GEMM psum) ----
                hprev = hh_r[:, s, :, :, 1]
                if lo == 0:
                    mv1 = hprev[:, 0, :]
                    nL1 = _nv(c, 0)
                    for g, dst in ((0, pt[:, 512 + 8 * t:512 + 8 * t + 8]),
                                   (1, pt[:, 640 + 8 * t:640 + 8 * t + 8]),
                                   (2, nL1[:, 8 * t:8 * t + 8, 0])):
                        nc.tensor.matmul(
                            dst, wh1[:, g * 128:(g + 1) * 128], mv1,
                            start=False, stop=True, skip_group_check=True)
                if hi == 2:
                    mv2 = hprev[0:H2, 1, :]
                    nL2 = _nv(c, 1)
                    for g, dst in ((0, pt[:, 768 + 8 * t:768 + 8 * t + 8]),
                                   (1, pt[:, 896 + 8 * t:896 + 8 * t + 8]),
                                   (2, nL2[:, 8 * t:8 * t + 8, 0])):
                        nc.tensor.matmul(
                            dst, wh2[:, g * 128:(g + 1) * 128], mv2,
                            start=False, stop=True, skip_group_check=True)

                # --- sigmoid r,z -> dz (r pairs cols 0:32, z 32:64) ----
                nc.scalar.activation(
                    dzr[:, lo:hi, :, :, 1], rzv[:, lo:hi, :, 8 * t:8 * t + 8],
                    AF.Sigmoid)

                # --- scanA: t2 = r*(hn + b_hhn) + xn (per layer, DVE)
                if lo == 0:
                    nc.vector.tensor_tensor_scan(
                        out=t2s[:, 0:16], data0=dz[:, 0:16],
                        data1=pt[:, 16 * t:16 * t + 16],
                        initial=0.0, op0=ALU.mult, op1=ALU.add)
                if hi == 2:
                    nc.vector.tensor_tensor_scan(
                        out=t2s[:, 16:32], data0=dz[:, 16:32],
                        data1=pt[:, 256 + 16 * t:256 + 16 * t + 16],
                        initial=0.0, op0=ALU.mult, op1=ALU.add)

                # --- omz = sigmoid(-z_pre) -> d0B odds ----------------
                nc.scalar.activation(
                    d0Br[:, lo:hi, :, 1], rzv[:, lo:hi, 1, 8 * t:8 * t + 8],
                    AF.Sigmoid, scale=-1.0)

                # --- zh = z*h on gpsimd -> d1B odds -------------------
                nc.gpsimd.tensor_tensor(
                    out=d1Br[:, lo:hi, :, 1], in0=dzr[:, lo:hi, 1, :, 1],
                    in1=hprev[:, lo:hi, :], op=ALU.mult)

                # --- tanh -> d1B evens --------------------------------
                nc.scalar.activation(
                    d1Br[:, lo:hi, :, 0], t2sr[:, lo:hi, :, 1], AF.Tanh)

                # --- scanB: h' = omz*n + zh ---------------------------
                nc.vector.tensor_tensor_scan(
                    out=hh[:, 32 * (s + 1) + 16 * lo:32 * (s + 1) + 16 * hi],
                    data0=d0B[:, 16 * lo:16 * hi], data1=d1B[:, 16 * lo:16 * hi],
                    initial=0.0, op0=ALU.mult, op1=ALU.add)

            # ---- prologue --------------------------------------------
            dma_xchunk(0)
            dma_xchunk(1)
            alloc_pt(0)
            for f in gemm1_thunks(0):
                f()

            # ---- main loop -------------------------------------------
            thunks = []
            for s in range(NROUND):
                if s % CHUNK == 0:
                    k = s // CHUNK
                    if k + 1 <= NCH + 1:
                        alloc_pt(k + 1)
                    if k + 1 < NCH:
                        thunks += gemm1_thunks(k + 1)
                    if 0 <= k - 1 < NCH:
                        thunks += gemm2_thunks(k - 1, first=(k + 1 >= NCH))
                    if k + 2 < NCH:
                        dma_xchunk(k + 2)
                round_step(s)
                for _ in range(4):
                    if thunks:
                        thunks.pop(0)()
            while thunks:
                thunks.pop(0)()

            # ---- fc head ---------------------------------------------
            h2fin = hh_r[0:H2, NROUND, 1, :, 1]            # [64, 8] bf16
            fcp = ppool.tile([B_LOC, 1], DT_F32, tag="fc", name="fcp", bufs=1)
            nc.tensor.matmul(fcp[:], h2fin, fw[:], start=True, stop=True,
                             skip_group_check=True)
            nc.scalar.activation(res[:], fcp[:], AF.Identity, bias=fb[:])
            nc.sync.dma_start(out=out[:], in_=res[:])

    nc.compile()
    return nc


_NC_CACHE = {}


def _get_nc():
    if "nc" not in _NC_CACHE:
        _NC_CACHE["nc"] = build_nc()
    return _NC_CACHE["nc"]


def _pad_gates(m, hin):
    """[3*H2, hin] torch-layout weight -> [hin, 3*128] bf16 stationary with
    zero padding in output channels 64..127 of each gate."""
    out = np.zeros((hin, 3 * 128), dtype=np.float32)
    for g in range(3):
        out[:, g * 128:g * 128 + H2] = m[g * H2:(g + 1) * H2, :].T
    return out.astype(BF16)


def _prep_maps(x, w_ih1, w_hh1, b_ih1, b_hh1, w_ih2, w_hh2, b_ih2, b_hh2,
               fc_w, fc_b):
    f32 = np.float32
    brow1 = np.concatenate([
        (b_ih1[:H1] + b_hh1[:H1]),
        (b_ih1[H1:2 * H1] + b_hh1[H1:2 * H1]),
        b_ih1[2 * H1:],                         # n gate: b_ih only
    ]).reshape(1, 3 * 128)
    brow2 = np.zeros((1, 3 * 128), dtype=f32)
    brow2[0, 0:H2] = b_ih2[:H2] + b_hh2[:H2]
    brow2[0, 128:128 + H2] = b_ih2[H2:2 * H2] + b_hh2[H2:2 * H2]
    brow2[0, 256:256 + H2] = b_ih2[2 * H2:]
    bn2row = np.zeros((1, 128), dtype=f32)
    bn2row[0, :H2] = b_hh2[2 * H2:]
    shared = {
        "wih1T": np.ascontiguousarray(w_ih1.T).astype(BF16),
        "whh1T": np.ascontiguousarray(
            np.concatenate([w_hh1[g * H1:(g + 1) * H1, :].T for g in range(3)],
                           axis=1)).astype(BF16),
        "wih2T": _pad_gates(w_ih2, H1),
        "whh2T": _pad_gates(w_hh2, H2),
        "brow1": brow1.astype(BF16),
        "brow2": brow2.astype(BF16),
        "bn1row": np.ascontiguousarray(
            b_hh1[2 * H1:].reshape(1, 128)).astype(BF16),
        "bn2row": bn2row.astype(BF16),
        "fcwT": np.ascontiguousarray(fc_w.reshape(1, H2).T).astype(BF16),
        "fcb": np.full((B_LOC, 1), float(fc_b.reshape(-1)[0]), dtype=f32),
    }
    maps = []
    for c in range(N_CORES):
        xc = x[c * B_LOC:(c + 1) * B_LOC]          # [B_LOC, T, F]
        xTc = np.ascontiguousarray(xc.transpose(2, 1, 0)).astype(BF16)
        maps.append({"xT": xTc, **shared})
    return maps


def run(inputs, trace=False):
    nc = _get_nc()
    maps = _prep_maps(**inputs)
    res = run_bass_kernel_spmd(nc, maps, list(range(N_CORES)), trace=trace)
    outs = [np.asarray(res.results[i]["out"], np.float32) for i in range(N_CORES)]
    full = np.concatenate(outs, axis=0)            # [64, 1]
    return full, res.exec_time_ns


def kernel(**inputs):
    inputs = {k: np.asarray(v, np.float32) for k, v in inputs.items()}
    out, _ = run(inputs, trace=False)
    return out
